# revision 2
# baseline (speedup 1.0000x reference)
"""Trainium2 Bass kernel: decode attention with a 32K KV cache.

Problem: x[32,1024] -> qkv proj (16 heads, dh=64) -> attention over
(32768 cached + 32 new) keys -> c_proj. Returns (out[32,1024],
present[2,16,32800,64]).

Sharding: 2 heads per core (tensor parallel over n_head=16).
Each core gets:
  kt    [128, 32768]  K^T for its 2 heads (d-major: rows 0:64 head A,
                      64:128 head B) -- stationary operand of the scores
                      matmuls, streamed in 4096-key chunks.
  vaug  [32768, 130]  V rows [v_h0(64) | 1 | v_h1(64) | 1]; the ones
                      columns make the PV matmul also produce the softmax
                      denominator. Rows are host-permuted so the per-chunk
                      DMA is 16.6KB-contiguous per partition.
  xT, wq/wk/wv (per-core columns of w_attn; wq pre-scaled by 1/8),
  bqkv, wp (per-core rows of w_proj), maskT (causal mask for the 32 new
  keys).
Each core returns its partial c_proj output po[32,1024] (contribution of
its 2 heads); the host sums the 8 partials and adds b_proj. No on-device
collective needed. knew/vnew [128,32] return the new k/v for `present`.

Device algorithm per head (S^T layout, no max-subtraction -- scores are
q.k/8 with |score| < ~2, exp is safe in fp32; masked new-block entries
are zeroed multiplicatively after exp):
  per 128-key group g: S^T[g] [128,32] = (K^T chunk).T @ qT   (PSUM)
  per 2048 keys: P^T = exp(S^T bank [128,512])                (ACT->SBUF)
  per group: out[32,65] += P^T[g].T @ [V|1][g]                (PSUM accum)
  epilogue: a = out[:, :64] / out[:, 64:65]; partial = a_merged @ wp.
"""

import numpy as np
from contextlib import ExitStack

import concourse.bass as bass
import concourse.tile as tile
from concourse import bacc, mybir
from concourse.bass_utils import run_bass_kernel_spmd
from concourse.masks import make_identity

N_EMBD = 1024
N_HEAD = 16
DH = 64
S = 32
PAST = 32768
NCORES = 8
HPC = N_HEAD // NCORES        # 2 heads per core
DPC = HPC * DH                # 128 dims per core
F32 = mybir.dt.float32
BF16 = mybir.dt.bfloat16

# tuning knobs
CHUNK = 4096                  # keys per DMA chunk
SGK = 2048                    # keys per PSUM scores bank (16 groups x 128)
KV_BUFS = 2                   # double-buffer the kv chunk tiles
PT_BUFS = 4
K_BF16 = False                # cast K (and q for scores) to bf16
V_BF16 = False                # cast V (and P^T) to bf16

_CACHE = {}


def _build():
    key = (CHUNK, SGK, KV_BUFS, PT_BUFS, K_BF16, V_BF16)
    if key in _CACHE:
        return _CACHE[key]

    NCHUNK = PAST // CHUNK
    NSG = CHUNK // SGK        # supergroups per chunk
    GPS = SGK // 128          # 128-key groups per supergroup
    KDT = BF16 if K_BF16 else F32
    VDT = BF16 if V_BF16 else F32

    nc = bacc.Bacc("TRN2", target_bir_lowering=False, debug=False,
                   enable_asserts=False, num_devices=NCORES)

    kt = nc.dram_tensor("kt", [DPC, PAST], KDT, kind="ExternalInput").ap()
    vaug = nc.dram_tensor("vaug", [PAST, 130], VDT, kind="ExternalInput").ap()
    xT = nc.dram_tensor("xT", [N_EMBD, S], F32, kind="ExternalInput").ap()
    wq = nc.dram_tensor("wq", [N_EMBD, DPC], F32, kind="ExternalInput").ap()
    wk = nc.dram_tensor("wk", [N_EMBD, DPC], F32, kind="ExternalInput").ap()
    wv = nc.dram_tensor("wv", [N_EMBD, DPC], F32, kind="ExternalInput").ap()
    bqkv = nc.dram_tensor("bqkv", [DPC, 3], F32, kind="ExternalInput").ap()
    wp = nc.dram_tensor("wp", [DPC, N_EMBD], F32, kind="ExternalInput").ap()
    maskT = nc.dram_tensor("maskT", [S, S], F32, kind="ExternalInput").ap()
    po = nc.dram_tensor("po", [S, N_EMBD], F32, kind="ExternalOutput").ap()
    knew = nc.dram_tensor("knew", [DPC, S], F32, kind="ExternalOutput").ap()
    vnew = nc.dram_tensor("vnew", [DPC, S], F32, kind="ExternalOutput").ap()

    EXP = mybir.ActivationFunctionType.Exp

    with tile.TileContext(nc) as tc, ExitStack() as ctx:
        const = ctx.enter_context(tc.tile_pool(name="const", bufs=1))
        kpool = ctx.enter_context(tc.tile_pool(name="kpool", bufs=KV_BUFS))
        vpool = ctx.enter_context(tc.tile_pool(name="vpool", bufs=KV_BUFS))
        ppool = ctx.enter_context(tc.tile_pool(name="ppool", bufs=PT_BUFS))
        spsum = ctx.enter_context(tc.tile_pool(name="spsum", bufs=4, space="PSUM"))
        opsum = ctx.enter_context(tc.tile_pool(name="opsum", bufs=1, space="PSUM"))
        mpsum = ctx.enter_context(tc.tile_pool(name="mpsum", bufs=2, space="PSUM"))

        # ---- constants / weights ----
        xT_sb = const.tile([128, 8, S], F32, name="xT_sb", tag="xT_sb")
        nc.sync.dma_start(xT_sb, xT.rearrange("(c p) s -> p c s", p=128))
        w_sb = {}
        for nm, w in (("wq", wq), ("wk", wk), ("wv", wv)):
            t = const.tile([128, 8, DPC], F32, name=f"{nm}_sb", tag=f"{nm}_sb")
            nc.sync.dma_start(t, w.rearrange("(c p) f -> p c f", p=128))
            w_sb[nm] = t
        b_sb = const.tile([DPC, 3], F32, name="b_sb", tag="b_sb")
        nc.sync.dma_start(b_sb, bqkv)
        mask_sb = const.tile([S, S], F32, name="mask_sb", tag="mask_sb")
        nc.sync.dma_start(mask_sb, maskT)
        wp_sb = const.tile([DPC, N_EMBD], F32, name="wp_sb", tag="wp_sb")
        nc.sync.dma_start(wp_sb, wp)
        ident = const.tile([128, 128], F32, name="ident", tag="ident")
        make_identity(nc, ident)

        # ---- qkv projection (transposed layout [f, s]) ----
        qkv = {}
        for bi, nm in enumerate(("wq", "wk", "wv")):
            ps = mpsum.tile([128, S], F32, name=f"qkvp_{nm}", tag="mp")
            for c in range(8):
                nc.tensor.matmul(ps, lhsT=w_sb[nm][:, c, :], rhs=xT_sb[:, c, :],
                                 start=(c == 0), stop=(c == 7))
            sb = const.tile([128, S], F32, name=f"{nm}t", tag=f"{nm}t")
            nc.vector.tensor_scalar_add(sb, ps, b_sb[:, bi:bi + 1])
            qkv[nm] = sb
        q_sb, k_sb, v_sb = qkv["wq"], qkv["wk"], qkv["wv"]

        nc.sync.dma_start(knew, k_sb)
        nc.sync.dma_start(vnew, v_sb)

        if K_BF16:
            q_mm = const.tile([128, S], BF16, name="q_bf", tag="q_bf")
            nc.vector.tensor_copy(out=q_mm, in_=q_sb)
            k_mm = const.tile([128, S], BF16, name="k_bf", tag="k_bf")
            nc.vector.tensor_copy(out=k_mm, in_=k_sb)
        else:
            q_mm, k_mm = q_sb, k_sb

        # ---- new-key V (transpose to [t', d] and add ones cols) ----
        vt_ps = mpsum.tile([S, 128], F32, name="vt_ps", tag="mp")
        nc.tensor.transpose(vt_ps, v_sb, ident)
        vaug_new = const.tile([S, 130], VDT, name="vaug_new", tag="vaug_new")
        nc.vector.memset(vaug_new, 1.0)
        nc.scalar.copy(vaug_new[:, 0:64], vt_ps[:, 0:64])
        nc.scalar.copy(vaug_new[:, 65:129], vt_ps[:, 64:128])

        # ---- attention accumulators [32, 65] per head ----
        out_ps = [opsum.tile([S, 65], F32, name=f"out_ps{h}", tag=f"out_ps{h}")
                  for h in range(HPC)]

        # new-key block: S_new^T [t', s], exp, mask, PV (starts accumulation)
        for h in range(HPC):
            hp = h * DH
            sn = mpsum.tile([S, S], F32, name=f"snew{h}", tag="mp")
            nc.tensor.matmul(sn, lhsT=k_mm[hp:hp + DH, :], rhs=q_mm[hp:hp + DH, :],
                             start=True, stop=True)
            pn = const.tile([S, S], VDT, name=f"pnew{h}", tag=f"pnew{h}")
            nc.scalar.activation(pn, sn, EXP)
            nc.vector.tensor_mul(out=pn, in0=pn, in1=mask_sb)
            nc.tensor.matmul(out_ps[h], lhsT=pn, rhs=vaug_new[:, h * 65:(h + 1) * 65],
                             start=True, stop=False)

        # ---- main loop over the KV cache ----
        for ci in range(NCHUNK):
            kt_sb = kpool.tile([128, CHUNK], KDT, name="kt_sb", tag="kt")
            nc.sync.dma_start(kt_sb, kt[:, ci * CHUNK:(ci + 1) * CHUNK])
            vc_sb = vpool.tile([128, CHUNK // 128, 130], VDT, name="vc_sb", tag="v")
            nc.sync.dma_start(
                vc_sb,
                vaug[ci * CHUNK:(ci + 1) * CHUNK, :].rearrange("(p g) f -> p g f", p=128))
            for sg in range(NSG):
                for h in range(HPC):
                    hp = h * DH
                    sps = spsum.tile([128, GPS * S], F32, name="sps", tag="sc")
                    for g in range(GPS):
                        off = sg * SGK + g * 128
                        nc.tensor.matmul(sps[:, g * S:(g + 1) * S],
                                         lhsT=kt_sb[hp:hp + DH, off:off + 128],
                                         rhs=q_mm[hp:hp + DH, :],
                                         start=True, stop=True)
                    pt = ppool.tile([128, GPS * S], VDT, name="pt", tag="pt")
                    nc.scalar.activation(pt, sps, EXP)
                    last_sg = (ci == NCHUNK - 1) and (sg == NSG - 1)
                    for g in range(GPS):
                        gi = sg * GPS + g
                        nc.tensor.matmul(out_ps[h],
                                         lhsT=pt[:, g * S:(g + 1) * S],
                                         rhs=vc_sb[:, gi, h * 65:(h + 1) * 65],
                                         start=False, stop=(last_sg and g == GPS - 1))

        # ---- epilogue: divide by denominator, merge heads, c_proj ----
        a_both = const.tile([S, 128], F32, name="a_both", tag="a_both")
        for h in range(HPC):
            rc = const.tile([S, 1], F32, name=f"recip{h}", tag=f"recip{h}")
            nc.vector.reciprocal(rc, out_ps[h][:, 64:65])
            nc.vector.tensor_scalar_mul(a_both[:, h * DH:(h + 1) * DH],
                                        out_ps[h][:, 0:DH], rc)
        at_ps = mpsum.tile([128, S], F32, name="at_ps", tag="mp")
        nc.tensor.transpose(at_ps, a_both, ident[0:S, 0:S])
        at_sb = const.tile([128, S], F32, name="at_sb", tag="at_sb")
        nc.scalar.copy(at_sb, at_ps)
        out_sb = const.tile([S, N_EMBD], F32, name="out_sb", tag="out_sb")
        for half in range(2):
            pp = mpsum.tile([S, 512], F32, name=f"proj{half}", tag="mp")
            nc.tensor.matmul(pp, lhsT=at_sb, rhs=wp_sb[:, half * 512:(half + 1) * 512],
                             start=True, stop=True)
            nc.scalar.copy(out_sb[:, half * 512:(half + 1) * 512], pp)
        nc.sync.dma_start(po, out_sb)

    nc.compile()
    _CACHE[key] = nc
    return nc


def _np_bf16(a):
    import ml_dtypes
    return np.asarray(a, np.float32).astype(ml_dtypes.bfloat16)


def make_in_maps(x, layer_past, w_attn, b_attn, w_proj):
    """Host-side sharding: per-core input dict."""
    x = np.ascontiguousarray(np.asarray(x, np.float32))
    layer_past = np.asarray(layer_past, np.float32)
    w_attn = np.asarray(w_attn, np.float32)
    b_attn = np.asarray(b_attn, np.float32)

    xT = np.ascontiguousarray(x.T)                      # [1024, 32]
    mask = (np.arange(S)[:, None] <= np.arange(S)[None, :]).astype(np.float32)
    in_maps = []
    for c in range(NCORES):
        h0 = HPC * c
        f0 = h0 * DH
        kp = layer_past[0, h0:h0 + HPC]                 # [2, 32768, 64]
        ktc = np.ascontiguousarray(
            kp.transpose(0, 2, 1).reshape(DPC, PAST))   # [128, 32768]
        vp = layer_past[1, h0:h0 + HPC]                 # [2, 32768, 64]
        va = np.ones((PAST, 130), np.float32)
        va[:, 0:64] = vp[0]
        va[:, 65:129] = vp[1]
        # permute rows so each partition's 32 rows per 4096-chunk are contiguous
        va = np.ascontiguousarray(
            va.reshape(PAST // CHUNK, CHUNK // 128, 128, 130)
              .transpose(0, 2, 1, 3).reshape(PAST, 130))
        wqc = np.ascontiguousarray(w_attn[:, f0:f0 + DPC]) / 8.0
        wkc = np.ascontiguousarray(w_attn[:, N_EMBD + f0:N_EMBD + f0 + DPC])
        wvc = np.ascontiguousarray(w_attn[:, 2 * N_EMBD + f0:2 * N_EMBD + f0 + DPC])
        bq = b_attn[f0:f0 + DPC] / 8.0
        bk = b_attn[N_EMBD + f0:N_EMBD + f0 + DPC]
        bv = b_attn[2 * N_EMBD + f0:2 * N_EMBD + f0 + DPC]
        bqkvc = np.ascontiguousarray(
            np.stack([bq, bk, bv], axis=1).astype(np.float32))
        wpc = np.ascontiguousarray(np.asarray(w_proj, np.float32)[f0:f0 + DPC])
        in_maps.append(dict(
            kt=_np_bf16(ktc) if K_BF16 else ktc,
            vaug=_np_bf16(va) if V_BF16 else va,
            xT=xT, wq=wqc.astype(np.float32), wk=wkc, wv=wvc,
            bqkv=bqkvc, wp=wpc, maskT=mask))
    return in_maps


def gather(results, layer_past, b_proj):
    """Host-side unshard: sum partials, assemble present."""
    layer_past = np.asarray(layer_past, np.float32)
    b_proj = np.asarray(b_proj, np.float32)
    out = np.zeros((S, N_EMBD), np.float32)
    k_new = np.empty((N_HEAD, S, DH), np.float32)
    v_new = np.empty((N_HEAD, S, DH), np.float32)
    for c in range(NCORES):
        out += results[c]["po"]
        kT = results[c]["knew"]                        # [128, 32]
        vT = results[c]["vnew"]
        for h in range(HPC):
            k_new[HPC * c + h] = kT[h * DH:(h + 1) * DH, :].T
            v_new[HPC * c + h] = vT[h * DH:(h + 1) * DH, :].T
    out = out + b_proj
    present = np.concatenate(
        [layer_past, np.stack([k_new, v_new], axis=0)], axis=2)
    return out.astype(np.float32), present.astype(np.float32)


def run(inputs, trace=False, **kw):
    """Build (cached), run on 8 cores, return (BassKernelResults, in_maps)."""
    nc = _build()
    in_maps = make_in_maps(inputs["x"], inputs["layer_past"], inputs["w_attn"],
                           inputs["b_attn"], inputs["w_proj"])
    res = run_bass_kernel_spmd(nc, in_maps, core_ids=list(range(NCORES)),
                               trace=trace, **kw)
    return res, in_maps


def kernel(x, layer_past, w_attn, b_attn, w_proj, b_proj, seq_len, past_len):
    assert int(seq_len) == S and int(past_len) == PAST
    inputs = dict(x=x, layer_past=layer_past, w_attn=w_attn, b_attn=b_attn,
                  w_proj=w_proj)
    res, _ = run(inputs)
    return gather(res.results, layer_past, b_proj)


# revision 7
# speedup vs baseline: 3.5206x; 3.5206x over previous
"""Trainium2 Bass kernel: decode attention with a 32K KV cache.

Problem: x[32,1024] -> qkv proj (16 heads, dh=64) -> attention over
(32768 cached + 32 new) keys -> c_proj. Returns (out[32,1024],
present[2,16,32800,64]).

Sharding: 2 heads per core (tensor parallel over n_head=16).
Each core gets:
  kt    [128, 32768]  K^T for its 2 heads (d-major: rows 0:64 head A,
                      64:128 head B) -- stationary operand of the scores
                      matmuls, streamed in 4096-key chunks.
  vaug  [32768, 130]  V rows [v_h0(64) | 1 | v_h1(64) | 1]; the ones
                      columns make the PV matmul also produce the softmax
                      denominator. Rows are host-permuted so the per-chunk
                      DMA is 16.6KB-contiguous per partition.
  xT, wq/wk/wv (per-core columns of w_attn; wq pre-scaled by 1/8),
  bqkv, wp (per-core rows of w_proj), maskT (causal mask for the 32 new
  keys).
Each core returns its partial c_proj output po[32,1024] (contribution of
its 2 heads); the host sums the 8 partials and adds b_proj. No on-device
collective needed. knew/vnew [128,32] return the new k/v for `present`.

Device algorithm per head (S^T layout, no max-subtraction -- scores are
q.k/8 with |score| < ~2, exp is safe in fp32; masked new-block entries
are zeroed multiplicatively after exp):
  per 128-key group g: S^T[g] [128,32] = (K^T chunk).T @ qT   (PSUM)
  per 2048 keys: P^T = exp(S^T bank [128,512])                (ACT->SBUF)
  per group: out[32,65] += P^T[g].T @ [V|1][g]                (PSUM accum)
  epilogue: a = out[:, :64] / out[:, 64:65]; partial = a_merged @ wp.
"""

import numpy as np
from contextlib import ExitStack

import concourse.bass as bass
import concourse.tile as tile
from concourse import bacc, mybir
from concourse.bass_utils import run_bass_kernel_spmd
from concourse.masks import make_identity

N_EMBD = 1024
N_HEAD = 16
DH = 64
S = 32
PAST = 32768
NCORES = 8
HPC = N_HEAD // NCORES        # 2 heads per core
DPC = HPC * DH                # 128 dims per core
F32 = mybir.dt.float32
BF16 = mybir.dt.bfloat16

# tuning knobs
CHUNK = 4096                  # keys per DMA chunk
SGK = 2048                    # keys per PSUM scores bank (16 groups x 128)
KV_BUFS = 2                   # double-buffer the kv chunk tiles
PT_BUFS = 4
K_BF16 = True                 # cast K (and q for scores) to bf16
V_BF16 = True                 # cast V (and P^T) to bf16
QKV_BF16 = True               # bf16 qkv-projection weights + xT

_CACHE = {}


def _build():
    key = (CHUNK, SGK, KV_BUFS, PT_BUFS, K_BF16, V_BF16, QKV_BF16)
    if key in _CACHE:
        return _CACHE[key]

    NCHUNK = PAST // CHUNK
    NSG = CHUNK // SGK        # supergroups per chunk
    GPS = SGK // 128          # 128-key groups per supergroup
    KDT = BF16 if K_BF16 else F32
    VDT = BF16 if V_BF16 else F32
    WDT = BF16 if QKV_BF16 else F32

    nc = bacc.Bacc("TRN2", target_bir_lowering=False, debug=False,
                   enable_asserts=False, num_devices=NCORES)

    kt = nc.dram_tensor("kt", [DPC, PAST], KDT, kind="ExternalInput").ap()
    vaug = nc.dram_tensor("vaug", [PAST, 130], VDT, kind="ExternalInput").ap()
    xT = nc.dram_tensor("xT", [N_EMBD, S], WDT, kind="ExternalInput").ap()
    wq = nc.dram_tensor("wq", [N_EMBD, DPC], WDT, kind="ExternalInput").ap()
    wk = nc.dram_tensor("wk", [N_EMBD, DPC], WDT, kind="ExternalInput").ap()
    wv = nc.dram_tensor("wv", [N_EMBD, DPC], WDT, kind="ExternalInput").ap()
    bqkv = nc.dram_tensor("bqkv", [DPC, 3], F32, kind="ExternalInput").ap()
    wp = nc.dram_tensor("wp", [DPC, N_EMBD], F32, kind="ExternalInput").ap()
    maskT = nc.dram_tensor("maskT", [S, S], VDT, kind="ExternalInput").ap()
    po = nc.dram_tensor("po", [S, N_EMBD], F32, kind="ExternalOutput").ap()
    knew = nc.dram_tensor("knew", [DPC, S], F32, kind="ExternalOutput").ap()
    vnew = nc.dram_tensor("vnew", [DPC, S], F32, kind="ExternalOutput").ap()

    EXP = mybir.ActivationFunctionType.Exp

    with tile.TileContext(nc) as tc, ExitStack() as ctx:
        const = ctx.enter_context(tc.tile_pool(name="const", bufs=1))
        kpool = ctx.enter_context(tc.tile_pool(name="kpool", bufs=KV_BUFS))
        vpool = ctx.enter_context(tc.tile_pool(name="vpool", bufs=KV_BUFS))
        ppool = ctx.enter_context(tc.tile_pool(name="ppool", bufs=PT_BUFS))
        spsum = ctx.enter_context(tc.tile_pool(name="spsum", bufs=4, space="PSUM"))
        opsum = ctx.enter_context(tc.tile_pool(name="opsum", bufs=1, space="PSUM"))
        mpsum = ctx.enter_context(tc.tile_pool(name="mpsum", bufs=2, space="PSUM"))

        # ---- constants / weights ----
        xT_sb = const.tile([128, 8, S], WDT, name="xT_sb", tag="xT_sb")
        nc.sync.dma_start(xT_sb, xT.rearrange("(c p) s -> p c s", p=128))
        w_sb = {}
        for nm, w in (("wq", wq), ("wk", wk), ("wv", wv)):
            t = const.tile([128, 8, DPC], WDT, name=f"{nm}_sb", tag=f"{nm}_sb")
            nc.sync.dma_start(t, w.rearrange("(c p) f -> p c f", p=128))
            w_sb[nm] = t
        b_sb = const.tile([DPC, 3], F32, name="b_sb", tag="b_sb")
        nc.sync.dma_start(b_sb, bqkv)
        mask_sb = const.tile([S, S], VDT, name="mask_sb", tag="mask_sb")
        nc.sync.dma_start(mask_sb, maskT)
        wp_sb = const.tile([DPC, N_EMBD], F32, name="wp_sb", tag="wp_sb")
        nc.sync.dma_start(wp_sb, wp)
        ident = const.tile([128, 128], F32, name="ident", tag="ident")
        make_identity(nc, ident)

        # ---- qkv projection (transposed layout [f, s]) ----
        qkv = {}
        for bi, nm in enumerate(("wq", "wk", "wv")):
            ps = mpsum.tile([128, S], F32, name=f"qkvp_{nm}", tag="mp")
            for c in range(8):
                nc.tensor.matmul(ps, lhsT=w_sb[nm][:, c, :], rhs=xT_sb[:, c, :],
                                 start=(c == 0), stop=(c == 7))
            sb = const.tile([128, S], F32, name=f"{nm}t", tag=f"{nm}t")
            nc.vector.tensor_scalar_add(sb, ps, b_sb[:, bi:bi + 1])
            qkv[nm] = sb
        q_sb, k_sb, v_sb = qkv["wq"], qkv["wk"], qkv["wv"]

        nc.sync.dma_start(knew, k_sb)
        nc.sync.dma_start(vnew, v_sb)

        if K_BF16:
            q_mm = const.tile([128, S], BF16, name="q_bf", tag="q_bf")
            nc.vector.tensor_copy(out=q_mm, in_=q_sb)
            k_mm = const.tile([128, S], BF16, name="k_bf", tag="k_bf")
            nc.vector.tensor_copy(out=k_mm, in_=k_sb)
        else:
            q_mm, k_mm = q_sb, k_sb

        # ---- new-key V (transpose to [t', d] and add ones cols) ----
        vt_ps = mpsum.tile([S, 128], F32, name="vt_ps", tag="mp")
        nc.tensor.transpose(vt_ps, v_sb, ident)
        vaug_new = const.tile([S, 130], VDT, name="vaug_new", tag="vaug_new")
        nc.vector.memset(vaug_new, 1.0)
        nc.scalar.copy(vaug_new[:, 0:64], vt_ps[:, 0:64])
        nc.scalar.copy(vaug_new[:, 65:129], vt_ps[:, 64:128])

        # ---- attention accumulators [32, 65] per head ----
        out_ps = [opsum.tile([S, 65], F32, name=f"out_ps{h}", tag=f"out_ps{h}")
                  for h in range(HPC)]

        # new-key block: S_new^T [t', s], exp, mask, PV (starts accumulation)
        for h in range(HPC):
            hp = h * DH
            sn = mpsum.tile([S, S], F32, name=f"snew{h}", tag="mp")
            nc.tensor.matmul(sn, lhsT=k_mm[hp:hp + DH, :], rhs=q_mm[hp:hp + DH, :],
                             start=True, stop=True)
            pn = const.tile([S, S], VDT, name=f"pnew{h}", tag=f"pnew{h}")
            nc.scalar.activation(pn, sn, EXP)
            nc.vector.tensor_mul(out=pn, in0=pn, in1=mask_sb)
            nc.tensor.matmul(out_ps[h], lhsT=pn, rhs=vaug_new[:, h * 65:(h + 1) * 65],
                             start=True, stop=False)

        # ---- main loop over the KV cache ----
        for ci in range(NCHUNK):
            kt_sb = kpool.tile([128, CHUNK], KDT, name="kt_sb", tag="kt")
            nc.sync.dma_start(kt_sb, kt[:, ci * CHUNK:(ci + 1) * CHUNK])
            vc_sb = vpool.tile([128, CHUNK // 128, 130], VDT, name="vc_sb", tag="v")
            nc.sync.dma_start(
                vc_sb,
                vaug[ci * CHUNK:(ci + 1) * CHUNK, :].rearrange("(p g) f -> p g f", p=128))
            for sg in range(NSG):
                for h in range(HPC):
                    hp = h * DH
                    sps = spsum.tile([128, GPS * S], F32, name="sps", tag="sc")
                    for g in range(GPS):
                        off = sg * SGK + g * 128
                        nc.tensor.matmul(sps[:, g * S:(g + 1) * S],
                                         lhsT=kt_sb[hp:hp + DH, off:off + 128],
                                         rhs=q_mm[hp:hp + DH, :],
                                         start=True, stop=True)
                    pt = ppool.tile([128, GPS * S], VDT, name="pt", tag="pt")
                    nc.scalar.activation(pt, sps, EXP)
                    last_sg = (ci == NCHUNK - 1) and (sg == NSG - 1)
                    for g in range(GPS):
                        gi = sg * GPS + g
                        nc.tensor.matmul(out_ps[h],
                                         lhsT=pt[:, g * S:(g + 1) * S],
                                         rhs=vc_sb[:, gi, h * 65:(h + 1) * 65],
                                         start=False, stop=(last_sg and g == GPS - 1))

        # ---- epilogue: divide by denominator, merge heads, c_proj ----
        a_both = const.tile([S, 128], F32, name="a_both", tag="a_both")
        for h in range(HPC):
            rc = const.tile([S, 1], F32, name=f"recip{h}", tag=f"recip{h}")
            nc.vector.reciprocal(rc, out_ps[h][:, 64:65])
            nc.vector.tensor_scalar_mul(a_both[:, h * DH:(h + 1) * DH],
                                        out_ps[h][:, 0:DH], rc)
        at_ps = mpsum.tile([128, S], F32, name="at_ps", tag="mp")
        nc.tensor.transpose(at_ps, a_both, ident[0:S, 0:S])
        at_sb = const.tile([128, S], F32, name="at_sb", tag="at_sb")
        nc.scalar.copy(at_sb, at_ps)
        out_sb = const.tile([S, N_EMBD], F32, name="out_sb", tag="out_sb")
        for half in range(2):
            pp = mpsum.tile([S, 512], F32, name=f"proj{half}", tag="mp")
            nc.tensor.matmul(pp, lhsT=at_sb, rhs=wp_sb[:, half * 512:(half + 1) * 512],
                             start=True, stop=True)
            nc.scalar.copy(out_sb[:, half * 512:(half + 1) * 512], pp)
        nc.sync.dma_start(po, out_sb)

    nc.compile()
    _CACHE[key] = nc
    return nc


def _np_bf16(a):
    import ml_dtypes
    return np.asarray(a, np.float32).astype(ml_dtypes.bfloat16)


def make_in_maps(x, layer_past, w_attn, b_attn, w_proj):
    """Host-side sharding: per-core input dict."""
    x = np.ascontiguousarray(np.asarray(x, np.float32))
    layer_past = np.asarray(layer_past, np.float32)
    w_attn = np.asarray(w_attn, np.float32)
    b_attn = np.asarray(b_attn, np.float32)

    xT = np.ascontiguousarray(x.T)                      # [1024, 32]
    mask = (np.arange(S)[:, None] <= np.arange(S)[None, :]).astype(np.float32)
    in_maps = []
    for c in range(NCORES):
        h0 = HPC * c
        f0 = h0 * DH
        kp = layer_past[0, h0:h0 + HPC]                 # [2, 32768, 64]
        ktc = np.ascontiguousarray(
            kp.transpose(0, 2, 1).reshape(DPC, PAST))   # [128, 32768]
        vp = layer_past[1, h0:h0 + HPC]                 # [2, 32768, 64]
        va = np.ones((PAST, 130), np.float32)
        va[:, 0:64] = vp[0]
        va[:, 65:129] = vp[1]
        # permute rows so each partition's 32 rows per 4096-chunk are contiguous
        va = np.ascontiguousarray(
            va.reshape(PAST // CHUNK, CHUNK // 128, 128, 130)
              .transpose(0, 2, 1, 3).reshape(PAST, 130))
        wqc = np.ascontiguousarray(w_attn[:, f0:f0 + DPC]) / 8.0
        wkc = np.ascontiguousarray(w_attn[:, N_EMBD + f0:N_EMBD + f0 + DPC])
        wvc = np.ascontiguousarray(w_attn[:, 2 * N_EMBD + f0:2 * N_EMBD + f0 + DPC])
        bq = b_attn[f0:f0 + DPC] / 8.0
        bk = b_attn[N_EMBD + f0:N_EMBD + f0 + DPC]
        bv = b_attn[2 * N_EMBD + f0:2 * N_EMBD + f0 + DPC]
        bqkvc = np.ascontiguousarray(
            np.stack([bq, bk, bv], axis=1).astype(np.float32))
        wpc = np.ascontiguousarray(np.asarray(w_proj, np.float32)[f0:f0 + DPC])
        wqc = wqc.astype(np.float32)
        in_maps.append(dict(
            kt=_np_bf16(ktc) if K_BF16 else ktc,
            vaug=_np_bf16(va) if V_BF16 else va,
            xT=_np_bf16(xT) if QKV_BF16 else xT,
            wq=_np_bf16(wqc) if QKV_BF16 else wqc,
            wk=_np_bf16(wkc) if QKV_BF16 else wkc,
            wv=_np_bf16(wvc) if QKV_BF16 else wvc,
            bqkv=bqkvc, wp=wpc,
            maskT=_np_bf16(mask) if V_BF16 else mask))
    return in_maps


def gather(results, layer_past, b_proj):
    """Host-side unshard: sum partials, assemble present."""
    layer_past = np.asarray(layer_past, np.float32)
    b_proj = np.asarray(b_proj, np.float32)
    out = np.zeros((S, N_EMBD), np.float32)
    k_new = np.empty((N_HEAD, S, DH), np.float32)
    v_new = np.empty((N_HEAD, S, DH), np.float32)
    for c in range(NCORES):
        out += results[c]["po"]
        kT = results[c]["knew"]                        # [128, 32]
        vT = results[c]["vnew"]
        for h in range(HPC):
            k_new[HPC * c + h] = kT[h * DH:(h + 1) * DH, :].T
            v_new[HPC * c + h] = vT[h * DH:(h + 1) * DH, :].T
    out = out + b_proj
    present = np.concatenate(
        [layer_past, np.stack([k_new, v_new], axis=0)], axis=2)
    return out.astype(np.float32), present.astype(np.float32)


def run(inputs, trace=False, **kw):
    """Build (cached), run on 8 cores, return (BassKernelResults, in_maps)."""
    nc = _build()
    in_maps = make_in_maps(inputs["x"], inputs["layer_past"], inputs["w_attn"],
                           inputs["b_attn"], inputs["w_proj"])
    res = run_bass_kernel_spmd(nc, in_maps, core_ids=list(range(NCORES)),
                               trace=trace, **kw)
    return res, in_maps


def kernel(x, layer_past, w_attn, b_attn, w_proj, b_proj, seq_len, past_len):
    assert int(seq_len) == S and int(past_len) == PAST
    inputs = dict(x=x, layer_past=layer_past, w_attn=w_attn, b_attn=b_attn,
                  w_proj=w_proj)
    res, _ = run(inputs)
    return gather(res.results, layer_past, b_proj)
